# revision 12
# baseline (speedup 1.0000x reference)
"""Gated multi-head self-attention on 8 Trainium2 NeuronCores via Bass/Tile.

Problem: B=2, S=2048, E=1024, H=16, D=64, zero additive mask, gate=ones.
Sharding: core c handles batch b=c//4 and heads [4*(c%4), 4*(c%4)+4).
Each core computes its 4 heads' gated attention partial sum [S, E]; the
host adds the 4 partials per batch.

Device-side layout (per core, all bf16 matmuls, fp32 PSUM accumulate):
  xt   [E, S]        X^T for this batch (host pre-transposed + bf16 cast)
  wq   [E, 256]      per-head Wq/sqrt(D) stacked on columns (hd = h*64+d)
  wk   [E, 256]      Wk stacked
  wv   [E, 256]      Wv stacked
  wo   [256, E]      Wo stacked on rows, pre-scaled by eff_gate/denom
  mask [128, S/128]  additive mask column-major by t-chunk
  out  [S, E] fp32   partial output

Pipeline: QK^T projections -> per head: scores^T [t,s] tiles (PE),
exp via ScalarE (mask folded in as per-partition bias), PV with an
appended ones-column in V giving softmax denominators for free,
per-partition normalize (DVE), PE transpose back to [hd, s], final
O-projection, DMA out.
"""

import math
import os

import numpy as np

B = 2
S = 2048
E = 1024
H = 16
D = 64
P = 128
GATE_EPS = 1e-4
N_CORES = 8
NH = 4  # heads per core
HDC = NH * D  # 256 stacked head-dim columns per core

_BUILT = {}


def _build(seq_len=S, sblk=None):
    """Build the single-core Bass program (same program on all 8 cores)."""
    import concourse.bass as bass
    import concourse.bacc as bacc
    import concourse.mybir as mybir
    import concourse.tile as tile
    from concourse.masks import make_identity
    from contextlib import ExitStack

    bf16 = mybir.dt.bfloat16
    fp32 = mybir.dt.float32
    AF = mybir.ActivationFunctionType

    Sl = seq_len
    if sblk is None:
        sblk = min(1024, Sl)
    SBLK = sblk
    NSB = Sl // SBLK
    TCH = Sl // P  # number of 128-row t-chunks
    KT_E = E // P  # k-tiles over the embedding contraction
    NSC = SBLK // P  # 128-col s-chunks per s-block

    nc = bacc.Bacc()
    xt = nc.dram_tensor("xt", [E, Sl], bf16, kind="ExternalInput")
    wq = nc.dram_tensor("wq", [E, HDC], bf16, kind="ExternalInput")
    wk = nc.dram_tensor("wk", [E, HDC], bf16, kind="ExternalInput")
    wv = nc.dram_tensor("wv", [E, HDC], bf16, kind="ExternalInput")
    wo = nc.dram_tensor("wo", [HDC, E], bf16, kind="ExternalInput")
    mask = nc.dram_tensor("mask", [P, TCH], fp32, kind="ExternalInput")
    out = nc.dram_tensor("out", [Sl, E], fp32, kind="ExternalOutput")

    with tile.TileContext(nc) as tc, ExitStack() as ctx:
        const = ctx.enter_context(tc.tile_pool(name="const", bufs=1))
        xt_sb = const.tile([P, KT_E, Sl], bf16, tag="xt")
        wq_sb = const.tile([P, KT_E, HDC], bf16, tag="wq")
        wk_sb = const.tile([P, KT_E, HDC], bf16, tag="wk")
        wv_sb = const.tile([P, KT_E, HDC], bf16, tag="wv")
        wo_sb = const.tile([P, HDC // P, E], bf16, tag="wo")
        mask_sb = const.tile([P, TCH], fp32, tag="mask")
        ident = const.tile([P, P], bf16, tag="ident")
        qt_sb = const.tile([P, HDC // P, Sl], bf16, tag="qt")
        kt_sb = const.tile([P, HDC // P, Sl], bf16, tag="kt")
        vt_sb = const.tile([P, TCH, NH, D + 1], bf16, tag="vt")
        ct_sb = const.tile([P, HDC // P, Sl], bf16, tag="ct")

        nc.sync.dma_start(xt_sb[:], xt.rearrange("(ko p) s -> p ko s", p=P))
        nc.sync.dma_start(wq_sb[:], wq.rearrange("(ko p) n -> p ko n", p=P))
        nc.sync.dma_start(wk_sb[:], wk.rearrange("(ko p) n -> p ko n", p=P))
        nc.sync.dma_start(wv_sb[:], wv.rearrange("(ko p) n -> p ko n", p=P))
        nc.sync.dma_start(wo_sb[:], wo.rearrange("(kt p) e -> p kt e", p=P))
        nc.sync.dma_start(mask_sb[:], mask[:])
        make_identity(nc, ident[:])
        nc.vector.memset(vt_sb[:, :, :, D : D + 1], 1.0)

        # ---- Q^T / K^T projections: out[hd, s] = sum_e W[e, hd] * X^T[e, s]
        with tc.tile_pool(name="proj_psum", bufs=2, space="PSUM") as pj:
            for w_sb, dst in ((wq_sb, qt_sb), (wk_sb, kt_sb)):
                for ht in range(HDC // P):
                    for blk in range(Sl // SBLK):
                        ps = pj.tile([P, SBLK], fp32, tag="qkproj")
                        for k in range(KT_E):
                            for sc0 in range(0, SBLK, 512):
                                sc1 = min(sc0 + 512, SBLK)
                                nc.tensor.matmul(
                                    ps[:, sc0:sc1],
                                    lhsT=w_sb[:, k, ht * P : (ht + 1) * P],
                                    rhs=xt_sb[:, k, blk * SBLK + sc0 : blk * SBLK + sc1],
                                    start=(k == 0),
                                    stop=(k == KT_E - 1),
                                )
                        nc.vector.tensor_copy(
                            out=dst[:, ht, blk * SBLK : (blk + 1) * SBLK], in_=ps[:]
                        )
            # ---- V projection: out[t, hd] = sum_e X^T[e, t] * Wv[e, hd]
            for tch in range(TCH):
                pv = pj.tile([P, HDC], fp32, tag="vproj")
                for k in range(KT_E):
                    nc.tensor.matmul(
                        pv[:],
                        lhsT=xt_sb[:, k, tch * P : (tch + 1) * P],
                        rhs=wv_sb[:, k, :],
                        start=(k == 0),
                        stop=(k == KT_E - 1),
                    )
                nc.vector.tensor_copy(
                    out=vt_sb[:, tch, :, 0:D],
                    in_=pv.rearrange("p (h d) -> p h d", d=D),
                )

        # ---- attention main loop
        with tc.tile_pool(name="cacc_psum", bufs=1, space="PSUM") as cacc_pool, \
             tc.tile_pool(name="scores_psum", bufs=2, space="PSUM") as sc_pool, \
             tc.tile_pool(name="tp_psum", bufs=2, space="PSUM") as tp_pool, \
             tc.tile_pool(name="pt_pool", bufs=8) as pt_pool, \
             tc.tile_pool(name="norm_pool", bufs=4) as norm_pool:
            for h in range(NH):
                poff = (h % 2) * D
                ktile = h // 2
                for sb in range(NSB):
                    # two PSUM accumulator tiles so no matmul group crosses a bank
                    cacc_a = cacc_pool.tile([P, NSC // 2, D + 1], fp32, tag="cacc_a")
                    cacc_b = cacc_pool.tile([P, NSC - NSC // 2, D + 1], fp32, tag="cacc_b")
                    # PSUM start=True zeroes a whole 2KB bank, clobbering sibling
                    # accumulation groups in the same bank; zero explicitly and
                    # accumulate with start=False instead.
                    nc.vector.memset(cacc_a[:], 0.0)
                    nc.vector.memset(cacc_b[:], 0.0)

                    def cacc_ap(scnk):
                        if scnk < NSC // 2:
                            return cacc_a[:, scnk, :]
                        return cacc_b[:, scnk - NSC // 2, :]

                    for t in range(TCH):
                        sc_ps = sc_pool.tile([P, SBLK], fp32, tag="scores")
                        lhsT = kt_sb[poff : poff + D, ktile, t * P : (t + 1) * P]
                        for sc0 in range(0, SBLK, 512):
                            sc1 = min(sc0 + 512, SBLK)
                            nc.tensor.matmul(
                                sc_ps[:, sc0:sc1],
                                lhsT=lhsT,
                                rhs=qt_sb[poff : poff + D, ktile, sb * SBLK + sc0 : sb * SBLK + sc1],
                                start=True,
                                stop=True,
                            )
                        pt = pt_pool.tile([P, SBLK], bf16, tag="pt")
                        nc.scalar.activation(
                            pt[:], sc_ps[:], AF.Exp, bias=mask_sb[:, t : t + 1], scale=1.0
                        )
                        for scnk in range(NSC):
                            nc.tensor.matmul(
                                cacc_ap(scnk),
                                lhsT=pt[:, scnk * P : (scnk + 1) * P],
                                rhs=vt_sb[:, t, h, :],
                                start=False,
                                stop=(t == TCH - 1),
                                skip_group_check=True,
                            )
                    # normalize by the ones-column denominator, transpose to [hd, s]
                    for scnk in range(NSC):
                        ca = cacc_ap(scnk)
                        recip = norm_pool.tile([P, 1], fp32, tag="recip")
                        nc.vector.reciprocal(recip[:], ca[:, D : D + 1])
                        cn = norm_pool.tile([P, D], bf16, tag="cn")
                        nc.vector.tensor_scalar_mul(cn[:], ca[:, 0:D], recip[:])
                        tp = tp_pool.tile([D, P], bf16, tag="tp")
                        nc.tensor.transpose(tp[:], cn[:], ident[:])
                        nc.vector.tensor_copy(
                            out=ct_sb[
                                poff : poff + D,
                                ktile,
                                sb * SBLK + scnk * P : sb * SBLK + (scnk + 1) * P,
                            ],
                            in_=tp[:],
                        )

        # ---- O projection: out[s, e] = sum_hd C^T[hd, s] * Wo[hd, e]
        with tc.tile_pool(name="o_psum", bufs=2, space="PSUM") as op_pool, \
             tc.tile_pool(name="out_pool", bufs=3) as out_pool:
            for schunk in range(Sl // P):
                po = op_pool.tile([P, E], fp32, tag="oproj")
                for kt2 in range(HDC // P):
                    for ec in range(E // 512):
                        nc.tensor.matmul(
                            po[:, ec * 512 : (ec + 1) * 512],
                            lhsT=ct_sb[:, kt2, schunk * P : (schunk + 1) * P],
                            rhs=wo_sb[:, kt2, ec * 512 : (ec + 1) * 512],
                            start=(kt2 == 0),
                            stop=(kt2 == HDC // P - 1),
                        )
                ob = out_pool.tile([P, E], fp32, tag="ob")
                nc.vector.tensor_copy(out=ob[:], in_=po[:])
                nc.sync.dma_start(out[schunk * P : (schunk + 1) * P, :], ob[:])

    nc.compile()
    return nc


def _get_built(seq_len=S):
    if seq_len not in _BUILT:
        _BUILT[seq_len] = _build(seq_len)
    return _BUILT[seq_len]


def _host_prep(hidden_states, attention_mask, W_q, W_k, W_v, W_o, gate, seq_len=S):
    import ml_dtypes

    bf16 = ml_dtypes.bfloat16
    hs = np.asarray(hidden_states, dtype=np.float32)
    am = np.asarray(attention_mask, dtype=np.float32)
    W_q = np.asarray(W_q, dtype=np.float32)
    W_k = np.asarray(W_k, dtype=np.float32)
    W_v = np.asarray(W_v, dtype=np.float32)
    W_o = np.asarray(W_o, dtype=np.float32)
    gate = np.asarray(gate, dtype=np.float32)

    eff_gate = np.where(gate >= GATE_EPS, gate, 0.0)
    active = float(np.sum(gate > GATE_EPS))
    denom = max(1.0, active / H) if active > 0 else 1.0

    scale = 1.0 / math.sqrt(D)
    # [H, E, D] -> [E, H*D] head-stacked
    wq_all = np.ascontiguousarray((W_q * scale).transpose(1, 0, 2).reshape(E, H * D)).astype(bf16)
    wk_all = np.ascontiguousarray(W_k.transpose(1, 0, 2).reshape(E, H * D)).astype(bf16)
    wv_all = np.ascontiguousarray(W_v.transpose(1, 0, 2).reshape(E, H * D)).astype(bf16)
    wo_scaled = (W_o * (eff_gate / denom)[:, None, None]).reshape(H * D, E).astype(bf16)

    in_maps = []
    for c in range(N_CORES):
        b = c // 4
        g = c % 4
        hd0 = g * NH * D
        xt_c = np.ascontiguousarray(hs[b, :seq_len].T).astype(bf16)  # [E, S]
        mask_c = np.ascontiguousarray(
            am[b, 0, 0, :seq_len].reshape(seq_len // P, P).T
        ).astype(np.float32)  # [128, TCH]
        in_maps.append(
            {
                "xt": xt_c,
                "wq": np.ascontiguousarray(wq_all[:, hd0 : hd0 + HDC]),
                "wk": np.ascontiguousarray(wk_all[:, hd0 : hd0 + HDC]),
                "wv": np.ascontiguousarray(wv_all[:, hd0 : hd0 + HDC]),
                "wo": np.ascontiguousarray(wo_scaled[hd0 : hd0 + HDC, :]),
                "mask": mask_c,
            }
        )
    return in_maps


LAST_RESULTS = None


def _ensure_ntff_hook():
    """Install the antenv.axon_hooks shim + ctypes NTFF hook if absent.

    The agent image's antenv package lacks axon_hooks, so bass_utils'
    trace=True path can't find the profile hook; recreate what
    trn_agent_boot would have registered.
    """
    import sys
    import types

    try:
        from antenv.axon_hooks import get_axon_ntff_profile_hook  # noqa: F401

        return
    except ImportError:
        pass
    mod = types.ModuleType("antenv.axon_hooks")
    state = {"hook": None}
    mod.set_axon_ntff_profile_hook = lambda h: state.__setitem__("hook", h)
    mod.get_axon_ntff_profile_hook = lambda: state["hook"]
    sys.modules["antenv.axon_hooks"] = mod
    try:
        import antenv

        antenv.axon_hooks = mod
    except ImportError:
        pass
    try:
        from trn_agent_boot.trn_boot import _ntff_profile_via_ctypes

        mod.set_axon_ntff_profile_hook(
            _ntff_profile_via_ctypes("/opt/axon/libaxon_pjrt.so")
        )
    except Exception:
        pass


def kernel(hidden_states, attention_mask, W_q, W_k, W_v, W_o, gate):
    global LAST_RESULTS
    from concourse.bass_utils import run_bass_kernel_spmd

    nc = _get_built(S)
    in_maps = _host_prep(hidden_states, attention_mask, W_q, W_k, W_v, W_o, gate)
    trace = bool(os.environ.get("BASS_TRACE"))
    if trace:
        _ensure_ntff_hook()
    res = run_bass_kernel_spmd(nc, in_maps, core_ids=list(range(N_CORES)), trace=trace)
    LAST_RESULTS = res

    out = np.zeros((B, S, E), dtype=np.float32)
    for c in range(N_CORES):
        out[c // 4] += np.asarray(res.results[c]["out"], dtype=np.float32)
    return out
